# revision 1
# baseline (speedup 1.0000x reference)
"""Trainium2 Bass kernel for causal multi-head attention (prefill).

Problem: x[2,2048,768], 12 heads x 64 dim, causal softmax(QK^T/8)V + out-proj.

Sharding (8 cores, no collectives): core c handles batch c//4 and head group
c%4 (3 heads).  Each core computes, for its batch b and heads hs:
    qT,kT = (Wq_hs @ x_b^T), (Wk_hs @ x_b^T)        [192, 2048] (transposed)
    v     = x_b @ Wv_hs^T                            [2048, 192+ones]
    expT  = exp(scoresT/8) masked causally           [kv, sq] per head
    ctxT_h = v_aug^T @ expT  (extra row = softmax denom via ones column)
    outT_partial = Wo[:,cols_hs] @ (ctxT/den)        [768, 2048]
Host sums the 4 partial outputs per batch and transposes back.

All matmuls run as float32r (full-rate fp32 on the PE at N>=256); every
tensor feeding a matmul is float32r end-to-end (walrus requires producers
to round to f32r).  Softmax skips the max-subtraction: scores/8 ~ N(0,1),
so exp stays in fp32 range.  Causal masking: below-diagonal blocks are
computed at partial width starting at the diagonal, the 128x128 diagonal
triangle is masked by multiplying with a host-provided 0/1 mask, and
above-diagonal regions are simply never computed nor read.
"""

import numpy as np

import concourse.bass as bass
import concourse.tile as tile
from concourse import bacc, mybir
from concourse.bass_utils import run_bass_kernel_spmd

F32 = mybir.dt.float32
F32R = mybir.dt.float32r

B, S, D = 2, 2048, 768
H, DH = 12, 64
HPC = 3                 # heads per core
GH = HPC * DH           # 192 head dims per core
NCORES = 8
KT = D // 128           # 6 contraction tiles for projections
NSQ = S // 512          # 4 sq blocks of 512
NKV = S // 128          # 16 kv tiles of 128
WJ = 1024               # exp/ctx window width
NJ = S // WJ            # 2 windows


def build():
    nc = bacc.Bacc("TRN2", target_bir_lowering=False, debug=False)

    xT = nc.dram_tensor("xT", [D, S], F32R, kind="ExternalInput")
    wq = nc.dram_tensor("wq", [D, GH], F32R, kind="ExternalInput")
    wk = nc.dram_tensor("wk", [D, GH], F32R, kind="ExternalInput")
    wv = nc.dram_tensor("wv", [D, 256], F32R, kind="ExternalInput")  # 192 + 64 pad
    wo = nc.dram_tensor("wo", [GH, D], F32R, kind="ExternalInput")
    tri = nc.dram_tensor("tri", [128, 128], F32R, kind="ExternalInput")
    onesd = nc.dram_tensor("onesd", [1, 64], F32R, kind="ExternalInput")
    outT = nc.dram_tensor("outT", [D, S], F32, kind="ExternalOutput")

    with tile.TileContext(nc) as tc, \
         nc.allow_low_precision(reason="fp32r tiles feeding fp32r matmuls"):
        with tc.tile_pool(name="sb", bufs=1) as sb, \
             tc.tile_pool(name="sbe", bufs=3) as sbe, \
             tc.tile_pool(name="sbo", bufs=2) as sbo, \
             tc.tile_pool(name="ps", bufs=2, space="PSUM") as ps, \
             tc.tile_pool(name="psc", bufs=1, space="PSUM") as psc:

            # ---- phase 0: load weights + x ----
            xsb = sb.tile([128, KT, S], F32R, tag="xsb")
            x_r = xT[:, :].rearrange("(k p) n -> p k n", p=128)
            for k in range(KT):
                nc.sync.dma_start(xsb[:, k, :], x_r[:, k, :])

            wq_sb = sb.tile([128, KT, GH], F32R, tag="wq")
            wk_sb = sb.tile([128, KT, GH], F32R, tag="wk")
            wv_sb = sb.tile([128, KT, 256], F32R, tag="wv")
            nc.sync.dma_start(wq_sb, wq[:, :].rearrange("(k p) m -> p k m", p=128))
            nc.sync.dma_start(wk_sb, wk[:, :].rearrange("(k p) m -> p k m", p=128))
            nc.sync.dma_start(wv_sb, wv[:, :].rearrange("(k p) m -> p k m", p=128))
            wo01_sb = sb.tile([128, D], F32R, tag="wo01")
            wo2_sb = sb.tile([64, D], F32R, tag="wo2")
            nc.sync.dma_start(wo01_sb, wo[0:128, :])
            nc.sync.dma_start(wo2_sb, wo[128:GH, :])
            tri_sb = sb.tile([128, 128], F32R, tag="tri")
            nc.sync.dma_start(tri_sb, tri[:, :])
            ones_sb = sb.tile([1, 64], F32R, tag="ones")
            nc.sync.dma_start(ones_sb, onesd[:, :])

            # ---- phase 1: projections ----
            # qT/kT: [192, S] as [128, 2, S] tiles (Mt0 = heads 0/1, Mt1 = head 2)
            qt_sb = sb.tile([128, 2, S], F32R, tag="qt")
            kt_sb = sb.tile([128, 2, S], F32R, tag="kt")
            for dst, wsb in ((qt_sb, wq_sb), (kt_sb, wk_sb)):
                for mt in range(2):          # 128 rows, then 64 rows
                    mp = 128 if mt == 0 else 64
                    for nt in range(NSQ):
                        pp = ps.tile([128, 512], F32, tag="sc", name="pp")
                        for k in range(KT):
                            nc.tensor.matmul(
                                pp[:mp, :],
                                wsb[:, k, mt * 128:mt * 128 + mp],
                                xsb[:, k, nt * 512:(nt + 1) * 512],
                                start=(k == 0), stop=(k == KT - 1))
                        nc.vector.tensor_copy(
                            dst[:mp, mt, nt * 512:(nt + 1) * 512], pp[:mp, :])

            # v_aug: [128, NKV, 195]; per kv tile: head h v at cols 65h..65h+63,
            # ones at col 65h+64 (written via ACT copy: tri*0 + 1)
            vaug = sb.tile([128, NKV, 195], F32R, tag="vaug")
            for h in range(HPC):
                nc.scalar.activation(
                    vaug[:, :, 65 * h + 64:65 * h + 65],
                    tri_sb[:, h * NKV:(h + 1) * NKV].rearrange(
                        "p (t c) -> p t c", c=1),
                    mybir.ActivationFunctionType.Copy, bias=1.0, scale=0.0)
            for i in range(NKV):
                pp = ps.tile([128, 256], F32, tag="sc", name="pp")
                for k in range(KT):
                    nc.tensor.matmul(
                        pp,
                        xsb[:, k, i * 128:(i + 1) * 128],
                        wv_sb[:, k, :],
                        start=(k == 0), stop=(k == KT - 1))
                nc.vector.tensor_copy(
                    vaug[:, i, :].rearrange("p (h c) -> p h c", c=65)[:, :, 0:64],
                    pp[:, 0:192].rearrange("p (h c) -> p h c", c=64))

            # ---- phase 2: attention ----
            # ctxT: heads 0/1 packed in one [128, S] tile (h1 via partition-
            # shifting sbuf-to-sbuf DMA), head 2 in its own [64, S] tile.
            ctxT01 = sb.tile([128, S], F32R, tag="ctxT01")
            ctxT2 = sb.tile([64, S], F32R, tag="ctxT2")
            for J in range(NJ):
                for h in range(HPC):
                    if h < 2:
                        def kslc(i, h=h):
                            return kt_sb[64 * h:64 * h + 64, 0, i * 128:(i + 1) * 128]

                        def qslc(c0, c1, h=h):
                            return qt_sb[64 * h:64 * h + 64, 0, c0:c1]
                    else:
                        def kslc(i):
                            return kt_sb[0:64, 1, i * 128:(i + 1) * 128]

                        def qslc(c0, c1):
                            return qt_sb[0:64, 1, c0:c1]

                    ctx_ps = psc.tile([65, WJ], F32, tag="ctx", name="ctx_ps")
                    imax = 8 * J + 7
                    for i in range(imax + 1):
                        d = 128 * i - WJ * J       # window col of diagonal start
                        col0 = max(0, d)
                        nb0 = max(0, d // 512)
                        spsum = ps.tile([128, WJ], F32, tag="sc", name="spsum")
                        for nb in range(nb0, 2):
                            s0 = max(nb * 512, col0)
                            nc.tensor.matmul(
                                spsum[:, s0:(nb + 1) * 512],
                                kslc(i),
                                qslc(WJ * J + s0, WJ * J + (nb + 1) * 512),
                                start=True, stop=True)
                        esb = sbe.tile([128, WJ], F32R, tag="exp", name="esb")
                        nc.scalar.activation(
                            esb[:, col0:WJ], spsum[:, col0:WJ],
                            mybir.ActivationFunctionType.Exp, scale=0.125)
                        if d >= 0:
                            nc.vector.tensor_mul(
                                esb[:, d:d + 128], esb[:, d:d + 128], tri_sb)
                        for nb in range(nb0, 2):
                            s0 = max(nb * 512, col0)
                            nc.tensor.matmul(
                                ctx_ps[:, s0:(nb + 1) * 512],
                                vaug[:, i, 65 * h:65 * h + 65],
                                esb[:, s0:(nb + 1) * 512],
                                start=(i == 0), stop=(i == 8 * J + 4 * nb + 3))
                    # normalize rows 0:64 by row 64 (softmax denominator)
                    h1tmp = (sbo.tile([64, WJ], F32R, tag="h1tmp", name="h1tmp")
                             if h == 1 else None)
                    for nb in range(2):
                        c0 = WJ * J + nb * 512
                        inv = sbo.tile([1, 512], F32R, tag="inv", name="inv")
                        nc.vector.reciprocal(
                            inv, ctx_ps[64:65, nb * 512:(nb + 1) * 512])
                        bps = ps.tile([64, 512], F32, tag="sc", name="bps")
                        nc.tensor.matmul(bps, ones_sb, inv, start=True, stop=True)
                        bsb = sbo.tile([64, 512], F32, tag="bsb", name="bsb")
                        nc.vector.tensor_copy(bsb, bps)
                        if h == 0:
                            dst = ctxT01[0:64, c0:c0 + 512]
                        elif h == 1:
                            dst = h1tmp[:, nb * 512:(nb + 1) * 512]
                        else:
                            dst = ctxT2[:, c0:c0 + 512]
                        nc.vector.tensor_mul(
                            dst, ctx_ps[0:64, nb * 512:(nb + 1) * 512], bsb)
                    if h == 1:
                        # partition-shift h1's ctxT into rows 64:128
                        nc.sync.dma_start(
                            ctxT01[64:128, WJ * J:WJ * (J + 1)], h1tmp)

                # ---- phase 3: out-projection for this window ----
                for j in (2 * J, 2 * J + 1):
                    for mt in range(6):
                        ops = ps.tile([128, 512], F32, tag="sc", name="ops")
                        nc.tensor.matmul(
                            ops, wo01_sb[:, mt * 128:(mt + 1) * 128],
                            ctxT01[:, j * 512:(j + 1) * 512],
                            start=True, stop=False)
                        nc.tensor.matmul(
                            ops, wo2_sb[:, mt * 128:(mt + 1) * 128],
                            ctxT2[:, j * 512:(j + 1) * 512],
                            start=False, stop=True)
                        osb = sbo.tile([128, 512], F32, tag="osb", name="osb")
                        nc.vector.tensor_copy(osb, ops)
                        nc.sync.dma_start(
                            outT[mt * 128:(mt + 1) * 128, j * 512:(j + 1) * 512],
                            osb)

    nc.compile()
    return nc


def shard_inputs(x, Wq, Wk, Wv, Wo):
    x = np.asarray(x, np.float32)
    tri = np.triu(np.ones((128, 128), np.float32))
    ones = np.ones((1, 64), np.float32)
    in_maps = []
    for c in range(NCORES):
        b, g = c // 4, c % 4
        rs = slice(GH * g, GH * g + GH)
        wv_t = np.concatenate(
            [np.ascontiguousarray(np.asarray(Wv, np.float32)[rs].T),
             np.zeros((D, 64), np.float32)], axis=1)
        in_maps.append({
            "xT": np.ascontiguousarray(x[b].T),
            "wq": np.ascontiguousarray(np.asarray(Wq, np.float32)[rs].T),
            "wk": np.ascontiguousarray(np.asarray(Wk, np.float32)[rs].T),
            "wv": wv_t,
            "wo": np.ascontiguousarray(np.asarray(Wo, np.float32)[:, rs].T),
            "tri": tri,
            "onesd": ones,
        })
    return in_maps


def assemble(results, bo):
    out = np.zeros((B, S, D), np.float32)
    for c in range(NCORES):
        out[c // 4] += results[c]["outT"].T
    return out + np.asarray(bo, np.float32)[None, None, :]


_NC = None


def kernel(x, Wq, Wk, Wv, Wo, bo, **run_kwargs):
    global _NC
    if _NC is None:
        _NC = build()
    in_maps = shard_inputs(x, Wq, Wk, Wv, Wo)
    res = run_bass_kernel_spmd(_NC, in_maps, core_ids=list(range(NCORES)),
                               **run_kwargs)
    out = assemble(res.results, bo)
    kernel.last_results = res
    return out



# revision 3
# speedup vs baseline: 1.1094x; 1.1094x over previous
"""Trainium2 Bass kernel for causal multi-head attention (prefill).

Problem: x[2,2048,768], 12 heads x 64 dim, causal softmax(QK^T/8)V + out-proj.

Sharding (8 cores, no collectives): core c handles batch c//4 and head group
c%4 (3 heads).  Each core computes, for its batch b and heads hs:
    qT,kT = (Wq_hs @ x_b^T), (Wk_hs @ x_b^T)        [192, 2048] (transposed)
    v     = x_b @ Wv_hs^T                            [2048, 192+ones]
    expT  = exp(scoresT/8) masked causally           [kv, sq] per head
    ctxT_h = v_aug^T @ expT  (extra row = softmax denom via ones column)
    outT_partial = Wo[:,cols_hs] @ (ctxT/den)        [768, 2048]
Host sums the 4 partial outputs per batch and transposes back.

v2 vs baseline (293us):
  - 512-wide q windows; the three heads' score matmuls are interleaved per
    kv-tile so adjacent PE matmuls sit on different row-groups (h0 rows
    0-63, h1 rows 64-127, h2 rows 0-63) and execute concurrently.
  - Softmax normalization rebuilt: reciprocal_approx_fast on the [1,512]
    denominator row, a K=1 PE broadcast matmul (emitted behind the next
    window's chains so it never stalls the PE), and one DVE multiply.
    The baseline's 3.4us 1-lane DVE reciprocals caused ~50us of PE idle
    and HAM re-throttling to 1.2 GHz after every (window, head).
  - Out-projection for window J is emitted after window J+1's chains so
    normalize results are always ready.
"""

import numpy as np

import concourse.bass as bass
import concourse.tile as tile
from concourse import bacc, mybir
from concourse.bass_utils import run_bass_kernel_spmd

F32 = mybir.dt.float32
F32R = mybir.dt.float32r

B, S, D = 2, 2048, 768
H, DH = 12, 64
HPC = 3                 # heads per core
GH = HPC * DH           # 192 head dims per core
NCORES = 8
KT = D // 128           # 6 contraction tiles for projections
WJ = 512                # q window width
NJ = S // WJ            # 4 windows
NKV = S // 128          # 16 kv tiles of 128


def build():
    nc = bacc.Bacc("TRN2", target_bir_lowering=False, debug=False)

    xT = nc.dram_tensor("xT", [D, S], F32R, kind="ExternalInput")
    wq = nc.dram_tensor("wq", [D, GH], F32R, kind="ExternalInput")
    wk = nc.dram_tensor("wk", [D, GH], F32R, kind="ExternalInput")
    wv = nc.dram_tensor("wv", [D, 256], F32R, kind="ExternalInput")  # 192 + 64 pad
    wo = nc.dram_tensor("wo", [GH, D], F32R, kind="ExternalInput")
    tri = nc.dram_tensor("tri", [128, 128], F32R, kind="ExternalInput")
    onesd = nc.dram_tensor("onesd", [1, 64], F32R, kind="ExternalInput")
    outT = nc.dram_tensor("outT", [D, S], F32, kind="ExternalOutput")

    with tile.TileContext(nc) as tc, \
         nc.allow_low_precision(reason="fp32r tiles feeding fp32r matmuls"):
        with tc.tile_pool(name="sb", bufs=1) as sb, \
             tc.tile_pool(name="sbe", bufs=6) as sbe, \
             tc.tile_pool(name="sbo", bufs=3) as sbo, \
             tc.tile_pool(name="sbn", bufs=2) as sbn, \
             tc.tile_pool(name="ps", bufs=4, space="PSUM") as ps, \
             tc.tile_pool(name="psc", bufs=3, space="PSUM") as psc:

            # ---- phase 0: load weights + x ----
            xsb = sb.tile([128, KT, S], F32R, tag="xsb")
            x_r = xT[:, :].rearrange("(k p) n -> p k n", p=128)
            nc.sync.dma_start(xsb[:, 0, :], x_r[:, 0, :])
            wq_sb = sb.tile([128, KT, GH], F32R, tag="wq")
            wk_sb = sb.tile([128, KT, GH], F32R, tag="wk")
            wv_sb = sb.tile([128, KT, 256], F32R, tag="wv")
            nc.sync.dma_start(wq_sb, wq[:, :].rearrange("(k p) m -> p k m", p=128))
            nc.sync.dma_start(wk_sb, wk[:, :].rearrange("(k p) m -> p k m", p=128))
            for k in range(1, KT):
                nc.sync.dma_start(xsb[:, k, :], x_r[:, k, :])
            nc.sync.dma_start(wv_sb, wv[:, :].rearrange("(k p) m -> p k m", p=128))
            wo01_sb = sb.tile([128, D], F32R, tag="wo01")
            wo2_sb = sb.tile([64, D], F32R, tag="wo2")
            nc.sync.dma_start(wo01_sb, wo[0:128, :])
            nc.sync.dma_start(wo2_sb, wo[128:GH, :])
            tri_sb = sb.tile([128, 128], F32R, tag="tri")
            nc.sync.dma_start(tri_sb, tri[:, :])
            # ones row parked at partition 64 so the denominator-broadcast
            # matmul's lhsT/rhs share a base partition (row-group 64).
            ones64 = sb.tile([65, 64], F32R, tag="ones64")
            nc.sync.dma_start(ones64[64:65, :], onesd[:, :])

            # ---- phase 1: projections ----
            # qT/kT: [192, S] as [128, 2, S] tiles (Mt0 = heads 0/1, Mt1 = head 2)
            qt_sb = sb.tile([128, 2, S], F32R, tag="qt")
            kt_sb = sb.tile([128, 2, S], F32R, tag="kt")

            def proj_qk(nt):
                for dst, wsb in ((qt_sb, wq_sb), (kt_sb, wk_sb)):
                    for mt in range(2):          # 128 rows, then 64 rows
                        mp = 128 if mt == 0 else 64
                        pp = ps.tile([128, 512], F32, tag="sc", name="pp")
                        for k in range(KT):
                            nc.tensor.matmul(
                                pp[:mp, :],
                                wsb[:, k, mt * 128:mt * 128 + mp],
                                xsb[:, k, nt * 512:(nt + 1) * 512],
                                start=(k == 0), stop=(k == KT - 1))
                        nc.vector.tensor_copy(
                            dst[:mp, mt, nt * 512:(nt + 1) * 512], pp[:mp, :])

            # v_aug: [128, NKV, 195]; per kv tile: head h v at cols 65h..65h+63,
            # ones at col 65h+64 (written via ACT copy: tri*0 + 1)
            vaug = sb.tile([128, NKV, 195], F32R, tag="vaug")
            for h in range(HPC):
                nc.scalar.activation(
                    vaug[:, :, 65 * h + 64:65 * h + 65],
                    tri_sb[:, h * NKV:(h + 1) * NKV].rearrange(
                        "p (t c) -> p t c", c=1),
                    mybir.ActivationFunctionType.Copy, bias=1.0, scale=0.0)

            def proj_v(i):
                pp = ps.tile([128, 256], F32, tag="sc", name="pp")
                for k in range(KT):
                    nc.tensor.matmul(
                        pp,
                        xsb[:, k, i * 128:(i + 1) * 128],
                        wv_sb[:, k, :],
                        start=(k == 0), stop=(k == KT - 1))
                nc.vector.tensor_copy(
                    vaug[:, i, :].rearrange("p (h c) -> p h c", c=65)[:, :, 0:64],
                    pp[:, 0:192].rearrange("p (h c) -> p h c", c=64))

            # ---- phase 2/3 helpers ----
            # head slicing: h0 = rows 0-63 of mt0, h1 = rows 64-127 of mt0,
            # h2 = rows 0-63 of mt1.  Adjacent scores matmuls alternate
            # row-groups (0, 64, 0) so consecutive PE matmuls overlap.
            def kslc(h, i):
                if h < 2:
                    return kt_sb[64 * h:64 * h + 64, 0, i * 128:(i + 1) * 128]
                return kt_sb[0:64, 1, i * 128:(i + 1) * 128]

            def qslc(h, c0, c1):
                if h < 2:
                    return qt_sb[64 * h:64 * h + 64, 0, c0:c1]
                return qt_sb[0:64, 1, c0:c1]

            ctxT01 = sb.tile([128, S], F32R, tag="ctxT01")
            ctxT2 = sb.tile([64, S], F32R, tag="ctxT2")

            def chains(J):
                """scores+exp+mask+ctx for window J, heads interleaved."""
                ctxp = [psc.tile([65, WJ], F32, tag="ctx", name=f"ctx{J}_{h}")
                        for h in range(HPC)]
                imax = 4 * J + 3
                for i in range(imax + 1):
                    d = 128 * i - WJ * J
                    col0 = max(0, d)
                    sps = []
                    for h in range(HPC):
                        spsum = ps.tile([128, WJ], F32, tag="sc", name="sp")
                        nc.tensor.matmul(
                            spsum[:, col0:WJ],
                            kslc(h, i),
                            qslc(h, WJ * J + col0, WJ * (J + 1)),
                            start=True, stop=True)
                        sps.append(spsum)
                    esbs = []
                    for h in range(HPC):
                        esb = sbe.tile([128, WJ], F32R, tag="exp", name="esb")
                        nc.scalar.activation(
                            esb[:, col0:WJ], sps[h][:, col0:WJ],
                            mybir.ActivationFunctionType.Exp, scale=0.125)
                        if d >= 0:
                            nc.vector.tensor_mul(
                                esb[:, d:d + 128], esb[:, d:d + 128], tri_sb)
                        esbs.append(esb)
                    for h in range(HPC):
                        nc.tensor.matmul(
                            ctxp[h][:, col0:WJ],
                            vaug[:, i, 65 * h:65 * h + 65],
                            esbs[h][:, col0:WJ],
                            start=(i == 0), stop=(i == imax))
                return ctxp

            def norms(J, ctxp):
                """normalize rows 0:64 by row 64 (softmax denominator).
                PE only runs the K=1 den-broadcast; recip + final mul on DVE.
                (walrus: a tensor op may read at most one PSUM operand, so
                the reciprocal lands in SBUF before the multiply.)"""
                h1t = None
                for h in range(HPC):
                    denr = sbn.tile([65, WJ], F32R, tag="denr", name="denr")
                    nc.vector.tensor_copy(denr[64:65, :], ctxp[h][64:65, :])
                    bps = ps.tile([64, WJ], F32, tag="sc", name="bps")
                    nc.tensor.matmul(bps, ones64[64:65, :], denr[64:65, :],
                                     start=True, stop=True)
                    invb = sbn.tile([64, WJ], F32, tag="invb", name="invb")
                    nc.vector.reciprocal_approx_fast(invb, bps)
                    if h == 0:
                        dst = ctxT01[0:64, WJ * J:WJ * (J + 1)]
                    elif h == 1:
                        h1t = sbo.tile([64, WJ], F32R, tag="h1t", name="h1t")
                        dst = h1t
                    else:
                        dst = ctxT2[:, WJ * J:WJ * (J + 1)]
                    nc.vector.tensor_mul(dst, ctxp[h][0:64, :], invb)
                # partition-shift h1's ctxT into rows 64:128
                nc.sync.dma_start(ctxT01[64:128, WJ * J:WJ * (J + 1)], h1t)

            def outproj(J):
                for mt in range(6):
                    ops = ps.tile([128, WJ], F32, tag="sc", name="ops")
                    nc.tensor.matmul(
                        ops, wo01_sb[:, mt * 128:(mt + 1) * 128],
                        ctxT01[:, J * WJ:(J + 1) * WJ],
                        start=True, stop=False)
                    nc.tensor.matmul(
                        ops, wo2_sb[:, mt * 128:(mt + 1) * 128],
                        ctxT2[:, J * WJ:(J + 1) * WJ],
                        start=False, stop=True)
                    osb = sbo.tile([128, WJ], F32, tag="osb", name="osb")
                    nc.vector.tensor_copy(osb, ops)
                    nc.sync.dma_start(
                        outT[mt * 128:(mt + 1) * 128, J * WJ:(J + 1) * WJ],
                        osb)

            # ---- interleaved schedule ----
            # proj nt / kv-tiles feed window J as soon as available; norms(J)
            # ride behind chains(J); outproj(J) behind chains(J+1).
            proj_qk(0)
            for i in range(4):
                proj_v(i)
            ctx0 = chains(0)
            proj_qk(1)
            for i in range(4, 8):
                proj_v(i)
            norms(0, ctx0)
            ctx1 = chains(1)
            proj_qk(2)
            for i in range(8, 12):
                proj_v(i)
            norms(1, ctx1)
            outproj(0)
            ctx2 = chains(2)
            proj_qk(3)
            for i in range(12, 16):
                proj_v(i)
            norms(2, ctx2)
            outproj(1)
            ctx3 = chains(3)
            norms(3, ctx3)
            outproj(2)
            outproj(3)

    nc.compile()
    return nc


def shard_inputs(x, Wq, Wk, Wv, Wo):
    x = np.asarray(x, np.float32)
    tri = np.triu(np.ones((128, 128), np.float32))
    ones = np.ones((1, 64), np.float32)
    in_maps = []
    for c in range(NCORES):
        b, g = c // 4, c % 4
        rs = slice(GH * g, GH * g + GH)
        wv_t = np.concatenate(
            [np.ascontiguousarray(np.asarray(Wv, np.float32)[rs].T),
             np.zeros((D, 64), np.float32)], axis=1)
        in_maps.append({
            "xT": np.ascontiguousarray(x[b].T),
            "wq": np.ascontiguousarray(np.asarray(Wq, np.float32)[rs].T),
            "wk": np.ascontiguousarray(np.asarray(Wk, np.float32)[rs].T),
            "wv": wv_t,
            "wo": np.ascontiguousarray(np.asarray(Wo, np.float32)[:, rs].T),
            "tri": tri,
            "onesd": ones,
        })
    return in_maps


def assemble(results, bo):
    out = np.zeros((B, S, D), np.float32)
    for c in range(NCORES):
        out[c // 4] += results[c]["outT"].T
    return out + np.asarray(bo, np.float32)[None, None, :]


_NC = None


def kernel(x, Wq, Wk, Wv, Wo, bo, **run_kwargs):
    global _NC
    if _NC is None:
        _NC = build()
    in_maps = shard_inputs(x, Wq, Wk, Wv, Wo)
    res = run_bass_kernel_spmd(_NC, in_maps, core_ids=list(range(NCORES)),
                               **run_kwargs)
    out = assemble(res.results, bo)
    kernel.last_results = res
    return out


# revision 4
# speedup vs baseline: 1.3204x; 1.1903x over previous
"""Trainium2 Bass kernel for causal multi-head attention (prefill).

Problem: x[2,2048,768], 12 heads x 64 dim, causal softmax(QK^T/8)V + out-proj.

Sharding (8 cores, no collectives): core c handles batch c//4 and head group
c%4 (3 heads).  Each core computes, for its batch b and heads hs:
    qT,kT = (Wq_hs @ x_b^T), (Wk_hs @ x_b^T)        [192, 2048] (transposed)
    v     = x_b @ Wv_hs^T                            [2048, 192+ones]
    expT  = exp(scoresT/8) masked causally           [kv, sq] per head
    ctxT_h = v_aug^T @ expT  (extra row = softmax denom via ones column)
    outT_partial = Wo[:,cols_hs] @ (ctxT/den)        [768, 2048]
Host sums the 4 partial outputs per batch and transposes back.

v3: full bf16 datapath (PSUM accumulation and softmax normalization stay
fp32).  fp32r matmuls must self-load their 4-byte weights inside the
matmul, which serializes a ~256-cycle weight load with every instruction
and defeats PE row-group concurrency; bf16 weights load via standalone
LDWEIGHTS that the PE pulls ahead, and adjacent matmuls on different
row-groups (h0 rows 0-63 / h1 rows 64-127) execute concurrently.
Host pre-rearranges weights so every DMA partition line is contiguous.
"""

import numpy as np

import concourse.bass as bass
import concourse.tile as tile
from concourse import bacc, mybir
from concourse.bass_utils import run_bass_kernel_spmd

F32 = mybir.dt.float32
BF16 = mybir.dt.bfloat16

B, S, D = 2, 2048, 768
H, DH = 12, 64
HPC = 3                 # heads per core
GH = HPC * DH           # 192 head dims per core
NCORES = 8
KT = D // 128           # 6 contraction tiles for projections
WJ = 512                # q window width
NJ = S // WJ            # 4 windows
NKV = S // 128          # 16 kv tiles of 128


def build():
    nc = bacc.Bacc("TRN2", target_bir_lowering=False, debug=False)

    # host pre-rearranged: line p of xr holds xT[k*128+p, :] for k=0..5, etc.
    xr = nc.dram_tensor("xr", [128, KT * S], BF16, kind="ExternalInput")
    wqr = nc.dram_tensor("wqr", [128, KT * GH], BF16, kind="ExternalInput")
    wkr = nc.dram_tensor("wkr", [128, KT * GH], BF16, kind="ExternalInput")
    wvr = nc.dram_tensor("wvr", [128, KT * 256], BF16, kind="ExternalInput")
    wo = nc.dram_tensor("wo", [GH, D], BF16, kind="ExternalInput")
    tri = nc.dram_tensor("tri", [128, 128], BF16, kind="ExternalInput")
    onesd = nc.dram_tensor("onesd", [1, 64], BF16, kind="ExternalInput")
    outT = nc.dram_tensor("outT", [D, S], F32, kind="ExternalOutput")

    with tile.TileContext(nc) as tc, \
         nc.allow_low_precision(reason="bf16 datapath, fp32 psum/normalize"):
        with tc.tile_pool(name="sb", bufs=1) as sb, \
             tc.tile_pool(name="sbe", bufs=6) as sbe, \
             tc.tile_pool(name="sbo", bufs=3) as sbo, \
             tc.tile_pool(name="sbn", bufs=2) as sbn, \
             tc.tile_pool(name="ps", bufs=4, space="PSUM") as ps, \
             tc.tile_pool(name="psc", bufs=3, space="PSUM") as psc:

            # ---- phase 0: load weights + x ----
            xsb = sb.tile([128, KT, S], BF16, tag="xsb")
            xr_v = xr[:, :].rearrange("p (k n) -> p k n", k=KT)
            nc.sync.dma_start(xsb[:, 0, :], xr_v[:, 0, :])
            wq_sb = sb.tile([128, KT, GH], BF16, tag="wq")
            wk_sb = sb.tile([128, KT, GH], BF16, tag="wk")
            wv_sb = sb.tile([128, KT, 256], BF16, tag="wv")
            nc.sync.dma_start(wq_sb, wqr[:, :].rearrange("p (k m) -> p k m", k=KT))
            nc.sync.dma_start(wk_sb, wkr[:, :].rearrange("p (k m) -> p k m", k=KT))
            for k in range(1, KT):
                nc.sync.dma_start(xsb[:, k, :], xr_v[:, k, :])
            nc.sync.dma_start(wv_sb, wvr[:, :].rearrange("p (k m) -> p k m", k=KT))
            wo01_sb = sb.tile([128, D], BF16, tag="wo01")
            wo2_sb = sb.tile([64, D], BF16, tag="wo2")
            nc.sync.dma_start(wo01_sb, wo[0:128, :])
            nc.sync.dma_start(wo2_sb, wo[128:GH, :])
            tri_sb = sb.tile([128, 128], BF16, tag="tri")
            nc.sync.dma_start(tri_sb, tri[:, :])
            # ones row parked at partition 64 so the denominator-broadcast
            # matmul's lhsT/rhs share a base partition (row-group 64).
            ones64 = sb.tile([65, 64], BF16, tag="ones64")
            nc.sync.dma_start(ones64[64:65, :], onesd[:, :])

            # ---- phase 1: projections ----
            # qT/kT: [192, S] as [128, 2, S] tiles (Mt0 = heads 0/1, Mt1 = head 2)
            qt_sb = sb.tile([128, 2, S], BF16, tag="qt")
            kt_sb = sb.tile([128, 2, S], BF16, tag="kt")

            def proj_qk(nt):
                for dst, wsb in ((qt_sb, wq_sb), (kt_sb, wk_sb)):
                    for mt in range(2):          # 128 rows, then 64 rows
                        mp = 128 if mt == 0 else 64
                        pp = ps.tile([128, 512], F32, tag="sc", name="pp")
                        for k in range(KT):
                            nc.tensor.matmul(
                                pp[:mp, :],
                                wsb[:, k, mt * 128:mt * 128 + mp],
                                xsb[:, k, nt * 512:(nt + 1) * 512],
                                start=(k == 0), stop=(k == KT - 1))
                        nc.vector.tensor_copy(
                            dst[:mp, mt, nt * 512:(nt + 1) * 512], pp[:mp, :])

            # v_aug: [128, NKV, 195]; per kv tile: head h v at cols 65h..65h+63,
            # ones at col 65h+64 (written via ACT copy: tri*0 + 1)
            vaug = sb.tile([128, NKV, 195], BF16, tag="vaug")
            for h in range(HPC):
                nc.scalar.activation(
                    vaug[:, :, 65 * h + 64:65 * h + 65],
                    tri_sb[:, h * NKV:(h + 1) * NKV].rearrange(
                        "p (t c) -> p t c", c=1),
                    mybir.ActivationFunctionType.Copy, bias=1.0, scale=0.0)

            def proj_v(i):
                pp = ps.tile([128, 256], F32, tag="sc", name="pp")
                for k in range(KT):
                    nc.tensor.matmul(
                        pp,
                        xsb[:, k, i * 128:(i + 1) * 128],
                        wv_sb[:, k, :],
                        start=(k == 0), stop=(k == KT - 1))
                nc.vector.tensor_copy(
                    vaug[:, i, :].rearrange("p (h c) -> p h c", c=65)[:, :, 0:64],
                    pp[:, 0:192].rearrange("p (h c) -> p h c", c=64))

            # ---- phase 2/3 helpers ----
            # head slicing: h0 = rows 0-63 of mt0, h1 = rows 64-127 of mt0,
            # h2 = rows 0-63 of mt1.  Adjacent scores matmuls alternate
            # row-groups (0, 64, 0) so consecutive PE matmuls overlap.
            def kslc(h, i):
                if h < 2:
                    return kt_sb[64 * h:64 * h + 64, 0, i * 128:(i + 1) * 128]
                return kt_sb[0:64, 1, i * 128:(i + 1) * 128]

            def qslc(h, c0, c1):
                if h < 2:
                    return qt_sb[64 * h:64 * h + 64, 0, c0:c1]
                return qt_sb[0:64, 1, c0:c1]

            ctxT01 = sb.tile([128, S], BF16, tag="ctxT01")
            ctxT2 = sb.tile([64, S], BF16, tag="ctxT2")

            def chains(J):
                """scores+exp+mask+ctx for window J, heads interleaved."""
                ctxp = [psc.tile([65, WJ], F32, tag="ctx", name=f"ctx{J}_{h}")
                        for h in range(HPC)]
                imax = 4 * J + 3
                for i in range(imax + 1):
                    d = 128 * i - WJ * J
                    col0 = max(0, d)
                    sps = []
                    for h in range(HPC):
                        spsum = ps.tile([128, WJ], F32, tag="sc", name="sp")
                        nc.tensor.matmul(
                            spsum[:, col0:WJ],
                            kslc(h, i),
                            qslc(h, WJ * J + col0, WJ * (J + 1)),
                            start=True, stop=True)
                        sps.append(spsum)
                    esbs = []
                    for h in range(HPC):
                        esb = sbe.tile([128, WJ], BF16, tag="exp", name="esb")
                        nc.scalar.activation(
                            esb[:, col0:WJ], sps[h][:, col0:WJ],
                            mybir.ActivationFunctionType.Exp, scale=0.125)
                        if d >= 0:
                            nc.vector.tensor_mul(
                                esb[:, d:d + 128], esb[:, d:d + 128], tri_sb)
                        esbs.append(esb)
                    for h in range(HPC):
                        nc.tensor.matmul(
                            ctxp[h][:, col0:WJ],
                            vaug[:, i, 65 * h:65 * h + 65],
                            esbs[h][:, col0:WJ],
                            start=(i == 0), stop=(i == imax))
                return ctxp

            def norms(J, ctxp):
                """normalize rows 0:64 by row 64 (softmax denominator).
                PE only runs the K=1 den-broadcast; recip + final mul on DVE.
                (walrus: a tensor op may read at most one PSUM operand, so
                the reciprocal lands in SBUF before the multiply.)"""
                h1t = None
                for h in range(HPC):
                    denr = sbn.tile([65, WJ], BF16, tag="denr", name="denr")
                    nc.vector.tensor_copy(denr[64:65, :], ctxp[h][64:65, :])
                    bps = ps.tile([64, WJ], F32, tag="sc", name="bps")
                    nc.tensor.matmul(bps, ones64[64:65, :], denr[64:65, :],
                                     start=True, stop=True)
                    invb = sbn.tile([64, WJ], F32, tag="invb", name="invb")
                    nc.vector.reciprocal_approx_fast(invb, bps)
                    if h == 0:
                        dst = ctxT01[0:64, WJ * J:WJ * (J + 1)]
                    elif h == 1:
                        h1t = sbo.tile([64, WJ], BF16, tag="h1t", name="h1t")
                        dst = h1t
                    else:
                        dst = ctxT2[:, WJ * J:WJ * (J + 1)]
                    nc.vector.tensor_mul(dst, ctxp[h][0:64, :], invb)
                # partition-shift h1's ctxT into rows 64:128
                nc.sync.dma_start(ctxT01[64:128, WJ * J:WJ * (J + 1)], h1t)

            def outproj(J):
                for mt in range(6):
                    ops = ps.tile([128, WJ], F32, tag="sc", name="ops")
                    nc.tensor.matmul(
                        ops, wo01_sb[:, mt * 128:(mt + 1) * 128],
                        ctxT01[:, J * WJ:(J + 1) * WJ],
                        start=True, stop=False)
                    nc.tensor.matmul(
                        ops, wo2_sb[:, mt * 128:(mt + 1) * 128],
                        ctxT2[:, J * WJ:(J + 1) * WJ],
                        start=False, stop=True)
                    osb = sbo.tile([128, WJ], F32, tag="osb", name="osb")
                    nc.vector.tensor_copy(osb, ops)
                    nc.sync.dma_start(
                        outT[mt * 128:(mt + 1) * 128, J * WJ:(J + 1) * WJ],
                        osb)

            # ---- interleaved schedule ----
            # proj nt / kv-tiles feed window J as soon as available; norms(J)
            # ride behind chains(J); outproj(J) behind chains(J+1).
            proj_qk(0)
            for i in range(4):
                proj_v(i)
            ctx0 = chains(0)
            proj_qk(1)
            for i in range(4, 8):
                proj_v(i)
            norms(0, ctx0)
            ctx1 = chains(1)
            proj_qk(2)
            for i in range(8, 12):
                proj_v(i)
            norms(1, ctx1)
            outproj(0)
            ctx2 = chains(2)
            proj_qk(3)
            for i in range(12, 16):
                proj_v(i)
            norms(2, ctx2)
            outproj(1)
            ctx3 = chains(3)
            norms(3, ctx3)
            outproj(2)
            outproj(3)

    nc.compile()
    return nc


def shard_inputs(x, Wq, Wk, Wv, Wo):
    import ml_dtypes
    bf16 = ml_dtypes.bfloat16

    def krearrange(wT, cols):
        # [D, cols] -> [128, KT*cols]; line p holds wT[k*128+p, :] for all k
        return np.ascontiguousarray(
            wT.reshape(KT, 128, cols).transpose(1, 0, 2).reshape(128, KT * cols)
        ).astype(bf16)

    x = np.asarray(x, np.float32)
    tri = np.triu(np.ones((128, 128), np.float32)).astype(bf16)
    ones = np.ones((1, 64), np.float32).astype(bf16)
    in_maps = []
    for c in range(NCORES):
        b, g = c // 4, c % 4
        rs = slice(GH * g, GH * g + GH)
        wv_t = np.concatenate(
            [np.ascontiguousarray(np.asarray(Wv, np.float32)[rs].T),
             np.zeros((D, 64), np.float32)], axis=1)
        in_maps.append({
            "xr": krearrange(np.ascontiguousarray(x[b].T), S),
            "wqr": krearrange(np.ascontiguousarray(np.asarray(Wq, np.float32)[rs].T), GH),
            "wkr": krearrange(np.ascontiguousarray(np.asarray(Wk, np.float32)[rs].T), GH),
            "wvr": krearrange(wv_t, 256),
            "wo": np.ascontiguousarray(np.asarray(Wo, np.float32)[:, rs].T).astype(bf16),
            "tri": tri,
            "onesd": ones,
        })
    return in_maps


def assemble(results, bo):
    out = np.zeros((B, S, D), np.float32)
    for c in range(NCORES):
        out[c // 4] += results[c]["outT"].T
    return out + np.asarray(bo, np.float32)[None, None, :]


_NC = None


def kernel(x, Wq, Wk, Wv, Wo, bo, **run_kwargs):
    global _NC
    if _NC is None:
        _NC = build()
    in_maps = shard_inputs(x, Wq, Wk, Wv, Wo)
    res = run_bass_kernel_spmd(_NC, in_maps, core_ids=list(range(NCORES)),
                               **run_kwargs)
    out = assemble(res.results, bo)
    kernel.last_results = res
    return out


# revision 5
# speedup vs baseline: 1.6501x; 1.2496x over previous
"""Trainium2 Bass kernel for causal multi-head attention (prefill).

Problem: x[2,2048,768], 12 heads x 64 dim, causal softmax(QK^T/8)V + out-proj.

Sharding (8 cores, no collectives): core c handles batch c//4 and head group
c%4 (3 heads).  Each core computes, for its batch b and heads hs:
    qT,kT = (Wq_hs @ x_b^T), (Wk_hs @ x_b^T)        [192, 2048] (transposed)
    v     = x_b @ Wv_hs^T                            [2048, 192+ones]
    expT  = exp(scoresT/8) masked causally           [kv, sq] per head
    ctxT_h = v_aug^T @ expT  (extra row = softmax denom via ones column)
    outT_partial = Wo[:,cols_hs] @ (ctxT/den)        [768, 2048]
Host sums the 4 partial outputs per batch and transposes back.

v4, built on hw microbenchmarks of the PE:
  - K=64 matmuls on alternating row-groups run 2x (concurrent tiles);
    weight reloads are free only for 128-column weights (FWL+background
    buffer), and a 65-column weight pays a ~120ns serial load.
  - So: head2's q/k live in BOTH row-group halves (dual copies via a
    partition-shift DMA) and the per-kv-tile score matmuls alternate
    groups perfectly: (g0,g64,g0) / (g64,g0,g64) by kv-tile parity.
  - ctx weights padded to 128 columns (vaug head stride 128, zero pad);
    output rows 65..127 of the ctx psum are dead.
  - Emission is software-pipelined (scores(i), exp(i), ctx(i-1)) so the
    Tile scheduler keeps score triplets adjacent (ctx emitted earlier
    would slot between them, breaking tile concurrency).
  - bf16 datapath; PSUM + softmax normalization fp32.
"""

import numpy as np

import concourse.bass as bass
import concourse.tile as tile
from concourse import bacc, mybir
from concourse.bass_utils import run_bass_kernel_spmd

F32 = mybir.dt.float32
BF16 = mybir.dt.bfloat16

B, S, D = 2, 2048, 768
H, DH = 12, 64
HPC = 3                 # heads per core
GH = HPC * DH           # 192 head dims per core
NCORES = 8
KT = D // 128           # 6 contraction tiles for projections
WJ = 512                # q window width
NJ = S // WJ            # 4 windows
NKV = S // 128          # 16 kv tiles of 128


def build():
    nc = bacc.Bacc("TRN2", target_bir_lowering=False, debug=False)

    # host pre-rearranged: line p holds tensor[k*128+p, :] slices for all k
    xr = nc.dram_tensor("xr", [128, KT * S], BF16, kind="ExternalInput")
    wqr = nc.dram_tensor("wqr", [128, KT * 128], BF16, kind="ExternalInput")
    wkr = nc.dram_tensor("wkr", [128, KT * 128], BF16, kind="ExternalInput")
    # packed mt1 weights: cols 0:64 = Wk2^T, cols 64:128 = Wq2^T
    wqk2r = nc.dram_tensor("wqk2r", [128, KT * 128], BF16, kind="ExternalInput")
    wvr = nc.dram_tensor("wvr", [128, KT * 256], BF16, kind="ExternalInput")
    wo = nc.dram_tensor("wo", [GH, D], BF16, kind="ExternalInput")
    tri = nc.dram_tensor("tri", [128, 128], BF16, kind="ExternalInput")
    onesd = nc.dram_tensor("onesd", [1, 64], BF16, kind="ExternalInput")
    outT = nc.dram_tensor("outT", [D, S], F32, kind="ExternalOutput")

    with tile.TileContext(nc) as tc, \
         nc.allow_low_precision(reason="bf16 datapath, fp32 psum/normalize"):
        with tc.tile_pool(name="sb", bufs=1) as sb, \
             tc.tile_pool(name="sbe", bufs=6) as sbe, \
             tc.tile_pool(name="sbo", bufs=3) as sbo, \
             tc.tile_pool(name="sbn", bufs=2) as sbn, \
             tc.tile_pool(name="ps", bufs=4, space="PSUM") as ps, \
             tc.tile_pool(name="psc", bufs=3, space="PSUM") as psc:

            # ---- phase 0: load weights + x ----
            xsb = sb.tile([128, KT, S], BF16, tag="xsb")
            xr_v = xr[:, :].rearrange("p (k n) -> p k n", k=KT)
            nc.sync.dma_start(xsb[:, 0, :], xr_v[:, 0, :])
            wq_sb = sb.tile([128, KT, 128], BF16, tag="wq")
            wk_sb = sb.tile([128, KT, 128], BF16, tag="wk")
            wqk2_sb = sb.tile([128, KT, 128], BF16, tag="wqk2")
            wv_sb = sb.tile([128, KT, 256], BF16, tag="wv")
            nc.sync.dma_start(wq_sb, wqr[:, :].rearrange("p (k m) -> p k m", k=KT))
            nc.sync.dma_start(wk_sb, wkr[:, :].rearrange("p (k m) -> p k m", k=KT))
            nc.sync.dma_start(wqk2_sb, wqk2r[:, :].rearrange("p (k m) -> p k m", k=KT))
            for k in range(1, KT):
                nc.sync.dma_start(xsb[:, k, :], xr_v[:, k, :])
            nc.sync.dma_start(wv_sb, wvr[:, :].rearrange("p (k m) -> p k m", k=KT))
            wo01_sb = sb.tile([128, D], BF16, tag="wo01")
            wo2_sb = sb.tile([64, D], BF16, tag="wo2")
            nc.sync.dma_start(wo01_sb, wo[0:128, :])
            nc.sync.dma_start(wo2_sb, wo[128:GH, :])
            tri_sb = sb.tile([128, 128], BF16, tag="tri")
            nc.sync.dma_start(tri_sb, tri[:, :])
            # ones row parked at partition 64 so the denominator-broadcast
            # matmul's lhsT/rhs share a base partition (row-group 64).
            ones64 = sb.tile([65, 64], BF16, tag="ones64")
            nc.sync.dma_start(ones64[64:65, :], onesd[:, :])

            # ---- phase 1: projections ----
            # heads 0/1: qt/kt [128, S] (h0 rows 0:64, h1 rows 64:128).
            # head 2: dual-group tiles q2d/k2d [128, S] - the same 64 rows
            # replicated in both halves so scores can alternate row-groups.
            qt_sb = sb.tile([128, S], BF16, tag="qt")
            kt_sb = sb.tile([128, S], BF16, tag="kt")
            q2d = sb.tile([128, S], BF16, tag="q2d")
            k2d = sb.tile([128, S], BF16, tag="k2d")

            def proj_qk(nt):
                ntw = slice(nt * 512, (nt + 1) * 512)
                for dst, wsb in ((qt_sb, wq_sb), (kt_sb, wk_sb)):
                    pp = ps.tile([128, 512], F32, tag="sc", name="pp")
                    for k in range(KT):
                        nc.tensor.matmul(pp, wsb[:, k, :], xsb[:, k, ntw],
                                         start=(k == 0), stop=(k == KT - 1))
                    nc.vector.tensor_copy(dst[:, ntw], pp)
                # packed mt1: psum rows 0:64 = k2, rows 64:128 = q2
                pp = ps.tile([128, 512], F32, tag="sc", name="pp")
                for k in range(KT):
                    nc.tensor.matmul(pp, wqk2_sb[:, k, :], xsb[:, k, ntw],
                                     start=(k == 0), stop=(k == KT - 1))
                nc.vector.tensor_copy(k2d[0:64, ntw], pp[0:64, :])
                nc.vector.tensor_copy(q2d[64:128, ntw], pp[64:128, :])
                # replicate into the other row-group half (partition shift)
                nc.sync.dma_start(k2d[64:128, ntw], k2d[0:64, ntw])
                nc.sync.dma_start(q2d[0:64, ntw], q2d[64:128, ntw])

            # v_aug: [128, NKV, 384]; head h: v at cols 128h..128h+63, ones
            # at col 128h+64, zeros 128h+65..128h+127 (128-col weight => FWL)
            vaug = sb.tile([128, NKV, 384], BF16, tag="vaug")
            for h in range(HPC):
                nc.vector.memset(vaug[:, :, 128 * h + 65:128 * (h + 1)], 0.0)
                nc.scalar.activation(
                    vaug[:, :, 128 * h + 64:128 * h + 65],
                    tri_sb[:, h * NKV:(h + 1) * NKV].rearrange(
                        "p (t c) -> p t c", c=1),
                    mybir.ActivationFunctionType.Copy, bias=1.0, scale=0.0)

            def proj_v(i):
                pp = ps.tile([128, 256], F32, tag="sc", name="pp")
                for k in range(KT):
                    nc.tensor.matmul(
                        pp,
                        xsb[:, k, i * 128:(i + 1) * 128],
                        wv_sb[:, k, :],
                        start=(k == 0), stop=(k == KT - 1))
                nc.vector.tensor_copy(
                    vaug[:, i, :].rearrange("p (h c) -> p h c", c=128)[:, :, 0:64],
                    pp[:, 0:192].rearrange("p (h c) -> p h c", c=64))

            # ---- phase 2/3 helpers ----
            # row-group per (head, kv-tile parity): h0 -> g0, h1 -> g64,
            # h2 -> g(i%2).  Emission order alternates groups exactly.
            def kslc(h, i):
                if h == 0:
                    return kt_sb[0:64, i * 128:(i + 1) * 128]
                if h == 1:
                    return kt_sb[64:128, i * 128:(i + 1) * 128]
                g = 64 * (i % 2)
                return k2d[g:g + 64, i * 128:(i + 1) * 128]

            def qslc(h, i, c0, c1):
                if h == 0:
                    return qt_sb[0:64, c0:c1]
                if h == 1:
                    return qt_sb[64:128, c0:c1]
                g = 64 * (i % 2)
                return q2d[g:g + 64, c0:c1]

            ctxT01 = sb.tile([128, S], BF16, tag="ctxT01")
            ctxT2 = sb.tile([64, S], BF16, tag="ctxT2")

            def chains(J):
                """scores+exp+mask for kv-tile i, ctx for i-1 (software
                pipeline keeps the score triplet adjacent on the PE queue)."""
                ctxp = [psc.tile([128, WJ], F32, tag="ctx", name=f"ctx{J}_{h}")
                        for h in range(HPC)]
                imax = 4 * J + 3

                def emit_ctx(i, i_esbs):
                    for h in range(HPC):
                        nc.tensor.matmul(
                            ctxp[h][:, max(0, 128 * i - WJ * J):WJ],
                            vaug[:, i, 128 * h:128 * h + 128],
                            i_esbs[h],
                            start=(i == 0), stop=(i == imax))

                prev = None
                for i in range(imax + 1):
                    d = 128 * i - WJ * J
                    col0 = max(0, d)
                    order = (0, 1, 2) if i % 2 == 0 else (1, 0, 2)
                    sps = {}
                    for h in order:
                        spsum = ps.tile([128, WJ], F32, tag="sc", name="sp")
                        nc.tensor.matmul(
                            spsum[:, col0:WJ],
                            kslc(h, i),
                            qslc(h, i, WJ * J + col0, WJ * (J + 1)),
                            start=True, stop=True)
                        sps[h] = spsum
                    esbs = []
                    for h in range(HPC):
                        esb = sbe.tile([128, WJ], BF16, tag="exp", name="esb")
                        nc.scalar.activation(
                            esb[:, col0:WJ], sps[h][:, col0:WJ],
                            mybir.ActivationFunctionType.Exp, scale=0.125)
                        if d >= 0:
                            nc.vector.tensor_mul(
                                esb[:, d:d + 128], esb[:, d:d + 128], tri_sb)
                        esbs.append(esb[:, col0:WJ])
                    if prev is not None:
                        emit_ctx(i - 1, prev)
                    prev = esbs
                emit_ctx(imax, prev)
                return ctxp

            def norms(J, ctxp):
                """normalize rows 0:64 by row 64 (softmax denominator).
                PE only runs the K=1 den-broadcast; recip + final mul on DVE.
                (walrus: a tensor op may read at most one PSUM operand, so
                the reciprocal lands in SBUF before the multiply.)"""
                h1t = None
                for h in range(HPC):
                    denr = sbn.tile([65, WJ], BF16, tag="denr", name="denr")
                    nc.vector.tensor_copy(denr[64:65, :], ctxp[h][64:65, :])
                    bps = ps.tile([64, WJ], F32, tag="sc", name="bps")
                    nc.tensor.matmul(bps, ones64[64:65, :], denr[64:65, :],
                                     start=True, stop=True)
                    invb = sbn.tile([64, WJ], F32, tag="invb", name="invb")
                    nc.vector.reciprocal_approx_fast(invb, bps)
                    if h == 0:
                        dst = ctxT01[0:64, WJ * J:WJ * (J + 1)]
                    elif h == 1:
                        h1t = sbo.tile([64, WJ], BF16, tag="h1t", name="h1t")
                        dst = h1t
                    else:
                        dst = ctxT2[:, WJ * J:WJ * (J + 1)]
                    nc.vector.tensor_mul(dst, ctxp[h][0:64, :], invb)
                # partition-shift h1's ctxT into rows 64:128
                nc.sync.dma_start(ctxT01[64:128, WJ * J:WJ * (J + 1)], h1t)

            def outproj(J):
                for mt in range(6):
                    ops = ps.tile([128, WJ], F32, tag="sc", name="ops")
                    nc.tensor.matmul(
                        ops, wo01_sb[:, mt * 128:(mt + 1) * 128],
                        ctxT01[:, J * WJ:(J + 1) * WJ],
                        start=True, stop=False)
                    nc.tensor.matmul(
                        ops, wo2_sb[:, mt * 128:(mt + 1) * 128],
                        ctxT2[:, J * WJ:(J + 1) * WJ],
                        start=False, stop=True)
                    osb = sbo.tile([128, WJ], F32, tag="osb", name="osb")
                    nc.vector.tensor_copy(osb, ops)
                    nc.sync.dma_start(
                        outT[mt * 128:(mt + 1) * 128, J * WJ:(J + 1) * WJ],
                        osb)

            # ---- interleaved schedule ----
            # proj nt / kv-tiles feed window J as soon as available; norms(J)
            # ride behind chains(J); outproj(J) behind chains(J+1).
            proj_qk(0)
            for i in range(4):
                proj_v(i)
            ctx0 = chains(0)
            proj_qk(1)
            for i in range(4, 8):
                proj_v(i)
            norms(0, ctx0)
            ctx1 = chains(1)
            proj_qk(2)
            for i in range(8, 12):
                proj_v(i)
            norms(1, ctx1)
            outproj(0)
            ctx2 = chains(2)
            proj_qk(3)
            for i in range(12, 16):
                proj_v(i)
            norms(2, ctx2)
            outproj(1)
            ctx3 = chains(3)
            norms(3, ctx3)
            outproj(2)
            outproj(3)

    nc.compile()
    return nc


def shard_inputs(x, Wq, Wk, Wv, Wo):
    import ml_dtypes
    bf16 = ml_dtypes.bfloat16

    def krearrange(wT, cols):
        # [D, cols] -> [128, KT*cols]; line p holds wT[k*128+p, :] for all k
        return np.ascontiguousarray(
            wT.reshape(KT, 128, cols).transpose(1, 0, 2).reshape(128, KT * cols)
        ).astype(bf16)

    x = np.asarray(x, np.float32)
    tri = np.triu(np.ones((128, 128), np.float32)).astype(bf16)
    ones = np.ones((1, 64), np.float32).astype(bf16)
    in_maps = []
    for c in range(NCORES):
        b, g = c // 4, c % 4
        rs = slice(GH * g, GH * g + GH)
        wqT = np.ascontiguousarray(np.asarray(Wq, np.float32)[rs].T)  # [D, 192]
        wkT = np.ascontiguousarray(np.asarray(Wk, np.float32)[rs].T)
        wqk2 = np.concatenate([wkT[:, 128:192], wqT[:, 128:192]], axis=1)
        wv_t = np.concatenate(
            [np.ascontiguousarray(np.asarray(Wv, np.float32)[rs].T),
             np.zeros((D, 64), np.float32)], axis=1)
        in_maps.append({
            "xr": krearrange(np.ascontiguousarray(x[b].T), S),
            "wqr": krearrange(np.ascontiguousarray(wqT[:, 0:128]), 128),
            "wkr": krearrange(np.ascontiguousarray(wkT[:, 0:128]), 128),
            "wqk2r": krearrange(np.ascontiguousarray(wqk2), 128),
            "wvr": krearrange(wv_t, 256),
            "wo": np.ascontiguousarray(np.asarray(Wo, np.float32)[:, rs].T).astype(bf16),
            "tri": tri,
            "onesd": ones,
        })
    return in_maps


def assemble(results, bo):
    out = np.zeros((B, S, D), np.float32)
    for c in range(NCORES):
        out[c // 4] += results[c]["outT"].T
    return out + np.asarray(bo, np.float32)[None, None, :]


_NC = None


def kernel(x, Wq, Wk, Wv, Wo, bo, **run_kwargs):
    global _NC
    if _NC is None:
        _NC = build()
    in_maps = shard_inputs(x, Wq, Wk, Wv, Wo)
    res = run_bass_kernel_spmd(_NC, in_maps, core_ids=list(range(NCORES)),
                               **run_kwargs)
    out = assemble(res.results, bo)
    kernel.last_results = res
    return out


# revision 6
# speedup vs baseline: 1.6933x; 1.0262x over previous
"""Trainium2 Bass kernel for causal multi-head attention (prefill).

Problem: x[2,2048,768], 12 heads x 64 dim, causal softmax(QK^T/8)V + out-proj.

Sharding (8 cores, no collectives): core c handles batch c//4 and head group
c%4 (3 heads).  Each core computes, for its batch b and heads hs:
    qT,kT = (Wq_hs @ x_b^T), (Wk_hs @ x_b^T)        [192, 2048] (transposed)
    v     = x_b @ Wv_hs^T                            [2048, 192+ones]
    expT  = exp(scoresT/8) masked causally           [kv, sq] per head
    ctxT_h = v_aug^T @ expT  (extra row = softmax denom via ones column)
    outT_partial = Wo[:,cols_hs] @ (ctxT/den)        [768, 2048]
Host sums the 4 partial outputs per batch and transposes back.

v4, built on hw microbenchmarks of the PE:
  - K=64 matmuls on alternating row-groups run 2x (concurrent tiles);
    weight reloads are free only for 128-column weights (FWL+background
    buffer), and a 65-column weight pays a ~120ns serial load.
  - So: head2's q/k live in BOTH row-group halves (dual copies via a
    partition-shift DMA) and the per-kv-tile score matmuls alternate
    groups perfectly: (g0,g64,g0) / (g64,g0,g64) by kv-tile parity.
  - ctx weights padded to 128 columns (vaug head stride 128, zero pad);
    output rows 65..127 of the ctx psum are dead.
  - Emission is software-pipelined (scores(i), exp(i), ctx(i-1)) so the
    Tile scheduler keeps score triplets adjacent (ctx emitted earlier
    would slot between them, breaking tile concurrency).
  - bf16 datapath; PSUM + softmax normalization fp32.
"""

import numpy as np

import concourse.bass as bass
import concourse.tile as tile
from concourse import bacc, mybir
from concourse.bass_utils import run_bass_kernel_spmd

F32 = mybir.dt.float32
BF16 = mybir.dt.bfloat16

B, S, D = 2, 2048, 768
H, DH = 12, 64
HPC = 3                 # heads per core
GH = HPC * DH           # 192 head dims per core
NCORES = 8
KT = D // 128           # 6 contraction tiles for projections
WJ = 512                # q window width
NJ = S // WJ            # 4 windows
NKV = S // 128          # 16 kv tiles of 128


def build():
    nc = bacc.Bacc("TRN2", target_bir_lowering=False, debug=False)

    # host pre-rearranged: line p holds tensor[k*128+p, :] slices for all k
    xr = nc.dram_tensor("xr", [128, KT * S], BF16, kind="ExternalInput")
    wqr = nc.dram_tensor("wqr", [128, KT * 128], BF16, kind="ExternalInput")
    wkr = nc.dram_tensor("wkr", [128, KT * 128], BF16, kind="ExternalInput")
    # packed mt1 weights: cols 0:64 = Wk2^T, cols 64:128 = Wq2^T
    wqk2r = nc.dram_tensor("wqk2r", [128, KT * 128], BF16, kind="ExternalInput")
    wvr = nc.dram_tensor("wvr", [128, KT * 256], BF16, kind="ExternalInput")
    wo = nc.dram_tensor("wo", [GH, D], BF16, kind="ExternalInput")
    tri = nc.dram_tensor("tri", [128, 128], BF16, kind="ExternalInput")
    onesd = nc.dram_tensor("onesd", [1, 64], BF16, kind="ExternalInput")
    outT = nc.dram_tensor("outT", [D, S], F32, kind="ExternalOutput")

    with tile.TileContext(nc) as tc, \
         nc.allow_low_precision(reason="bf16 datapath, fp32 psum/normalize"):
        with tc.tile_pool(name="sb", bufs=1) as sb, \
             tc.tile_pool(name="sbe", bufs=6) as sbe, \
             tc.tile_pool(name="sbo", bufs=3) as sbo, \
             tc.tile_pool(name="sbn", bufs=2) as sbn, \
             tc.tile_pool(name="ps", bufs=2, space="PSUM") as ps, \
             tc.tile_pool(name="psp", bufs=3, space="PSUM") as psp, \
             tc.tile_pool(name="psc", bufs=3, space="PSUM") as psc:

            # ---- phase 0: load weights + x ----
            xsb = sb.tile([128, KT, S], BF16, tag="xsb")
            xr_v = xr[:, :].rearrange("p (k n) -> p k n", k=KT)
            nc.sync.dma_start(xsb[:, 0, :], xr_v[:, 0, :])
            wq_sb = sb.tile([128, KT, 128], BF16, tag="wq")
            wk_sb = sb.tile([128, KT, 128], BF16, tag="wk")
            wqk2_sb = sb.tile([128, KT, 128], BF16, tag="wqk2")
            wv_sb = sb.tile([128, KT, 256], BF16, tag="wv")
            nc.sync.dma_start(wq_sb, wqr[:, :].rearrange("p (k m) -> p k m", k=KT))
            nc.sync.dma_start(wk_sb, wkr[:, :].rearrange("p (k m) -> p k m", k=KT))
            nc.sync.dma_start(wqk2_sb, wqk2r[:, :].rearrange("p (k m) -> p k m", k=KT))
            for k in range(1, KT):
                nc.sync.dma_start(xsb[:, k, :], xr_v[:, k, :])
            nc.sync.dma_start(wv_sb, wvr[:, :].rearrange("p (k m) -> p k m", k=KT))
            wo01_sb = sb.tile([128, D], BF16, tag="wo01")
            wo2_sb = sb.tile([64, D], BF16, tag="wo2")
            nc.sync.dma_start(wo01_sb, wo[0:128, :])
            nc.sync.dma_start(wo2_sb, wo[128:GH, :])
            tri_sb = sb.tile([128, 128], BF16, tag="tri")
            nc.sync.dma_start(tri_sb, tri[:, :])
            # ones row parked at partition 64 so the denominator-broadcast
            # matmul's lhsT/rhs share a base partition (row-group 64).
            ones64 = sb.tile([65, 64], BF16, tag="ones64")
            nc.sync.dma_start(ones64[64:65, :], onesd[:, :])

            # ---- phase 1: projections ----
            # heads 0/1: qt/kt [128, S] (h0 rows 0:64, h1 rows 64:128).
            # head 2: dual-group tiles q2d/k2d [128, S] - the same 64 rows
            # replicated in both halves so scores can alternate row-groups.
            qt_sb = sb.tile([128, S], BF16, tag="qt")
            kt_sb = sb.tile([128, S], BF16, tag="kt")
            q2d = sb.tile([128, S], BF16, tag="q2d")
            k2d = sb.tile([128, S], BF16, tag="k2d")

            def proj_qk(nt):
                ntw = slice(nt * 512, (nt + 1) * 512)
                for dst, wsb in ((qt_sb, wq_sb), (kt_sb, wk_sb)):
                    pp = ps.tile([128, 512], F32, tag="sc", name="pp")
                    for k in range(KT):
                        nc.tensor.matmul(pp, wsb[:, k, :], xsb[:, k, ntw],
                                         start=(k == 0), stop=(k == KT - 1))
                    nc.vector.tensor_copy(dst[:, ntw], pp)
                # packed mt1: psum rows 0:64 = k2, rows 64:128 = q2
                pp = ps.tile([128, 512], F32, tag="sc", name="pp")
                for k in range(KT):
                    nc.tensor.matmul(pp, wqk2_sb[:, k, :], xsb[:, k, ntw],
                                     start=(k == 0), stop=(k == KT - 1))
                nc.vector.tensor_copy(k2d[0:64, ntw], pp[0:64, :])
                nc.vector.tensor_copy(q2d[64:128, ntw], pp[64:128, :])
                # replicate into the other row-group half (partition shift)
                nc.sync.dma_start(k2d[64:128, ntw], k2d[0:64, ntw])
                nc.sync.dma_start(q2d[0:64, ntw], q2d[64:128, ntw])

            # v_aug: [128, NKV, 384]; head h: v at cols 128h..128h+63, ones
            # at col 128h+64, zeros 128h+65..128h+127 (128-col weight => FWL)
            vaug = sb.tile([128, NKV, 384], BF16, tag="vaug")
            for h in range(HPC):
                nc.vector.memset(vaug[:, :, 128 * h + 65:128 * (h + 1)], 0.0)
                nc.scalar.activation(
                    vaug[:, :, 128 * h + 64:128 * h + 65],
                    tri_sb[:, h * NKV:(h + 1) * NKV].rearrange(
                        "p (t c) -> p t c", c=1),
                    mybir.ActivationFunctionType.Copy, bias=1.0, scale=0.0)

            def proj_v(i):
                pp = ps.tile([128, 256], F32, tag="sc", name="pp")
                for k in range(KT):
                    nc.tensor.matmul(
                        pp,
                        xsb[:, k, i * 128:(i + 1) * 128],
                        wv_sb[:, k, :],
                        start=(k == 0), stop=(k == KT - 1))
                nc.vector.tensor_copy(
                    vaug[:, i, :].rearrange("p (h c) -> p h c", c=128)[:, :, 0:64],
                    pp[:, 0:192].rearrange("p (h c) -> p h c", c=64))

            # ---- phase 2/3 helpers ----
            # row-group per (head, kv-tile parity): h0 -> g0, h1 -> g64,
            # h2 -> g(i%2).  Emission order alternates groups exactly.
            def kslc(h, i):
                if h == 0:
                    return kt_sb[0:64, i * 128:(i + 1) * 128]
                if h == 1:
                    return kt_sb[64:128, i * 128:(i + 1) * 128]
                g = 64 * (i % 2)
                return k2d[g:g + 64, i * 128:(i + 1) * 128]

            def qslc(h, i, c0, c1):
                if h == 0:
                    return qt_sb[0:64, c0:c1]
                if h == 1:
                    return qt_sb[64:128, c0:c1]
                g = 64 * (i % 2)
                return q2d[g:g + 64, c0:c1]

            ctxT01 = sb.tile([128, S], BF16, tag="ctxT01")
            ctxT2 = sb.tile([64, S], BF16, tag="ctxT2")

            def chains(J):
                """scores+exp+mask for kv-tile i, ctx for i-1 (software
                pipeline keeps the score triplet adjacent on the PE queue)."""
                ctxp = [psc.tile([128, WJ], F32, tag="ctx", name=f"ctx{J}_{h}")
                        for h in range(HPC)]
                imax = 4 * J + 3

                def emit_ctx(i, i_esbs):
                    for h in range(HPC):
                        nc.tensor.matmul(
                            ctxp[h][:, max(0, 128 * i - WJ * J):WJ],
                            vaug[:, i, 128 * h:128 * h + 128],
                            i_esbs[h],
                            start=(i == 0), stop=(i == imax))

                prev = None
                for i in range(imax + 1):
                    d = 128 * i - WJ * J
                    col0 = max(0, d)
                    order = (0, 1, 2) if i % 2 == 0 else (1, 0, 2)
                    sps = {}
                    for h in order:
                        spsum = psp.tile([128, WJ], F32, tag="sp", name="sp")
                        nc.tensor.matmul(
                            spsum[:, col0:WJ],
                            kslc(h, i),
                            qslc(h, i, WJ * J + col0, WJ * (J + 1)),
                            start=True, stop=True)
                        sps[h] = spsum
                    esbs = []
                    for h in range(HPC):
                        esb = sbe.tile([128, WJ], BF16, tag="exp", name="esb")
                        nc.scalar.activation(
                            esb[:, col0:WJ], sps[h][:, col0:WJ],
                            mybir.ActivationFunctionType.Exp, scale=0.125)
                        if d >= 0:
                            nc.vector.tensor_mul(
                                esb[:, d:d + 128], esb[:, d:d + 128], tri_sb)
                        esbs.append(esb[:, col0:WJ])
                    if prev is not None:
                        emit_ctx(i - 1, prev)
                    prev = esbs
                emit_ctx(imax, prev)
                return ctxp

            def norms(J, ctxp):
                """normalize rows 0:64 by row 64 (softmax denominator).
                PE only runs the K=1 den-broadcast; recip + final mul on DVE.
                (walrus: a tensor op may read at most one PSUM operand, so
                the reciprocal lands in SBUF before the multiply.)"""
                h1t = None
                for h in range(HPC):
                    denr = sbn.tile([65, WJ], BF16, tag="denr", name="denr")
                    nc.vector.tensor_copy(denr[64:65, :], ctxp[h][64:65, :])
                    bps = ps.tile([64, WJ], F32, tag="sc", name="bps")
                    nc.tensor.matmul(bps, ones64[64:65, :], denr[64:65, :],
                                     start=True, stop=True)
                    invb = sbn.tile([64, WJ], F32, tag="invb", name="invb")
                    nc.vector.reciprocal_approx_fast(invb, bps)
                    if h == 0:
                        dst = ctxT01[0:64, WJ * J:WJ * (J + 1)]
                    elif h == 1:
                        h1t = sbo.tile([64, WJ], BF16, tag="h1t", name="h1t")
                        dst = h1t
                    else:
                        dst = ctxT2[:, WJ * J:WJ * (J + 1)]
                    nc.vector.tensor_mul(dst, ctxp[h][0:64, :], invb)
                # partition-shift h1's ctxT into rows 64:128
                nc.sync.dma_start(ctxT01[64:128, WJ * J:WJ * (J + 1)], h1t)

            def outproj(J):
                for mt in range(6):
                    ops = ps.tile([128, WJ], F32, tag="sc", name="ops")
                    nc.tensor.matmul(
                        ops, wo01_sb[:, mt * 128:(mt + 1) * 128],
                        ctxT01[:, J * WJ:(J + 1) * WJ],
                        start=True, stop=False)
                    nc.tensor.matmul(
                        ops, wo2_sb[:, mt * 128:(mt + 1) * 128],
                        ctxT2[:, J * WJ:(J + 1) * WJ],
                        start=False, stop=True)
                    osb = sbo.tile([128, WJ], F32, tag="osb", name="osb")
                    nc.vector.tensor_copy(osb, ops)
                    nc.sync.dma_start(
                        outT[mt * 128:(mt + 1) * 128, J * WJ:(J + 1) * WJ],
                        osb)

            # ---- interleaved schedule ----
            # proj nt / kv-tiles feed window J as soon as available; norms(J)
            # ride behind chains(J); outproj(J) behind chains(J+1).
            proj_qk(0)
            for i in range(4):
                proj_v(i)
            ctx0 = chains(0)
            proj_qk(1)
            for i in range(4, 8):
                proj_v(i)
            norms(0, ctx0)
            ctx1 = chains(1)
            proj_qk(2)
            for i in range(8, 12):
                proj_v(i)
            norms(1, ctx1)
            outproj(0)
            ctx2 = chains(2)
            proj_qk(3)
            for i in range(12, 16):
                proj_v(i)
            norms(2, ctx2)
            outproj(1)
            ctx3 = chains(3)
            norms(3, ctx3)
            outproj(2)
            outproj(3)

    nc.compile()
    return nc


def shard_inputs(x, Wq, Wk, Wv, Wo):
    import ml_dtypes
    bf16 = ml_dtypes.bfloat16

    def krearrange(wT, cols):
        # [D, cols] -> [128, KT*cols]; line p holds wT[k*128+p, :] for all k
        return np.ascontiguousarray(
            wT.reshape(KT, 128, cols).transpose(1, 0, 2).reshape(128, KT * cols)
        ).astype(bf16)

    x = np.asarray(x, np.float32)
    tri = np.triu(np.ones((128, 128), np.float32)).astype(bf16)
    ones = np.ones((1, 64), np.float32).astype(bf16)
    in_maps = []
    for c in range(NCORES):
        b, g = c // 4, c % 4
        rs = slice(GH * g, GH * g + GH)
        wqT = np.ascontiguousarray(np.asarray(Wq, np.float32)[rs].T)  # [D, 192]
        wkT = np.ascontiguousarray(np.asarray(Wk, np.float32)[rs].T)
        wqk2 = np.concatenate([wkT[:, 128:192], wqT[:, 128:192]], axis=1)
        wv_t = np.concatenate(
            [np.ascontiguousarray(np.asarray(Wv, np.float32)[rs].T),
             np.zeros((D, 64), np.float32)], axis=1)
        in_maps.append({
            "xr": krearrange(np.ascontiguousarray(x[b].T), S),
            "wqr": krearrange(np.ascontiguousarray(wqT[:, 0:128]), 128),
            "wkr": krearrange(np.ascontiguousarray(wkT[:, 0:128]), 128),
            "wqk2r": krearrange(np.ascontiguousarray(wqk2), 128),
            "wvr": krearrange(wv_t, 256),
            "wo": np.ascontiguousarray(np.asarray(Wo, np.float32)[:, rs].T).astype(bf16),
            "tri": tri,
            "onesd": ones,
        })
    return in_maps


def assemble(results, bo):
    out = np.zeros((B, S, D), np.float32)
    for c in range(NCORES):
        out[c // 4] += results[c]["outT"].T
    return out + np.asarray(bo, np.float32)[None, None, :]


_NC = None


def kernel(x, Wq, Wk, Wv, Wo, bo, **run_kwargs):
    global _NC
    if _NC is None:
        _NC = build()
    in_maps = shard_inputs(x, Wq, Wk, Wv, Wo)
    res = run_bass_kernel_spmd(_NC, in_maps, core_ids=list(range(NCORES)),
                               **run_kwargs)
    out = assemble(res.results, bo)
    kernel.last_results = res
    return out
